# revision 25
# baseline (speedup 1.0000x reference)
"""Bass/Trainium2 kernel for nn_BayesianBertSelfAttention (B=2,S=1024,HID=768,NH=12,HD=64).

Sharding: 24 (batch, head) pairs over 8 cores -> core c handles batch c//4,
heads {3k, 3k+1, 3k+2} with k = c%4.

Per-core device algorithm (transposed-scores layout, scoresT[r, l]):
  phase P: q/k projections as 4 column-packed matmul groups (qT/kT [64, S]
           bf16), v projections directly in [r, d] layout (bf16, with a ones
           column producing softmax row sums via the context matmul).
  phase R (pipelined into S): relative-position table R'[l, c] = q . E_rev
           band per 128-row l-tile (bf16 matmul), copied bf16 to a DRAM
           scratch with row pitch 1152.
  phase S: per head: skewed bf16 read of R' gives bias[l, r] tiles (the
           Music-Transformer skew as a strided DRAM access pattern); PE
           transposes them to [r, l] in a bf16 psum; one DVE add fuses
           psum scores + bias -> bf16 SBUF. Dual softmax: ACT exp(scale=1/8)
           over a fused [128, 2048] global|local pair (local pre-multiplied
           by SM^T on GPSIMD). Unnormalized bf16 probs feed two context
           matmuls; the ones-column of v gives row sums in row 64.
  phase F: PE transposes ctxT back to [l, d], DVE normalizes (reciprocal of
           row sums), blends the two branches with selector weights, DMA out.

Host: packs weights/binds layouts, converts to bf16, reassembles [2,1024,768].
"""

import sys

sys.path.insert(0, "/opt/trn_rl_repo")

import numpy as np
import ml_dtypes
from contextlib import ExitStack

import concourse.bass as bass
import concourse.bacc as bacc
import concourse.tile as tile
from concourse import mybir
from concourse.bass_utils import run_bass_kernel_spmd
from concourse.masks import make_identity

B, S, HID, NH, HD = 2, 1024, 768, 12, 64
MAXP = 1024
NCORES = 8
HPC = 3            # heads per core
LTN = S // 128     # 8 l-tiles
BAND = 1151        # skew band width per 128-row l-tile
PITCH = 1152       # skew scratch row pitch
NE = 2 * MAXP - 1  # 2047

BF16 = mybir.dt.bfloat16
F32 = mybir.dt.float32
FP8 = mybir.dt.float8e4
COPY = mybir.ActivationFunctionType.Copy
EXP = mybir.ActivationFunctionType.Exp

NPBF16 = ml_dtypes.bfloat16

_programs = {}


def _bcast(ap, dim_count, insert_at):
    """Insert a step-0 broadcast dim of size dim_count at free-dim position."""
    new = list(ap.ap)
    new.insert(insert_at, [0, dim_count])
    return bass.AP(tensor=ap.tensor, offset=ap.offset, ap=new)


def build_program(n_cc=6, use_m=False):
    """n_cc: number of 128-row contraction chunks for projections (6 normally,
    7 when biases are nonzero and folded in as an extra ones row)."""
    nc = bacc.Bacc(None)
    CH = n_cc * 128

    hidT = nc.dram_tensor("hidT", [CH, S], BF16, kind="ExternalInput")
    wg = nc.dram_tensor("wg", [4, CH, 128], BF16, kind="ExternalInput")
    wv = nc.dram_tensor("wv", [CH, HPC * HD], BF16, kind="ExternalInput")
    embT2 = nc.dram_tensor("embT2", [128, NE], BF16, kind="ExternalInput")
    smT = nc.dram_tensor("smT", [S, S], BF16, kind="ExternalInput")
    selw = nc.dram_tensor("selw", [S, 2], F32, kind="ExternalInput")
    if use_m:
        mvec = nc.dram_tensor("mvec", [S, 2], F32, kind="ExternalInput")  # [m, 8m]
    outp = nc.dram_tensor("out", [S, HPC * HD], F32, kind="ExternalOutput")
    skews = [nc.dram_tensor(f"skew{h}", [LTN * 128 * PITCH], FP8)
             for h in range(HPC)]

    # (q_h, k_h) SBUF partition offsets per head; G-groups: 0=[q0|q1], 1=[k0|k1],
    # 2=[q2|-], 3=[k2|-]
    QG = [(0, 0), (0, 64), (2, 0)]   # (group, partition offset) for q
    KG = [(1, 0), (1, 64), (3, 0)]

    with tile.TileContext(nc) as tc, ExitStack() as ctx:
        singles = ctx.enter_context(tc.tile_pool(name="singles", bufs=1))

        hid_sb = singles.tile([128, n_cc, S], BF16)
        wg_sb = singles.tile([128, 4, n_cc, 128], BF16)
        wv_sb = singles.tile([128, n_cc, HPC * HD], BF16)
        emb_sb = singles.tile([128, NE], BF16)
        smT_sb = singles.tile([128, 8, S], BF16)
        selw_sb = singles.tile([128, 8, 2], F32)
        hid_v = hidT.rearrange("(cc p) l -> p cc l", p=128)
        wg_v = wg.rearrange("g (cc p) d -> p g cc d", p=128)
        nc.sync.dma_start(out=wg_sb[:, 0], in_=wg_v[:, 0])
        for cc in range(n_cc):
            nc.sync.dma_start(out=hid_sb[:, cc], in_=hid_v[:, cc])
        for g in range(1, 4):
            nc.sync.dma_start(out=wg_sb[:, g], in_=wg_v[:, g])
        nc.sync.dma_start(out=emb_sb, in_=embT2[:, :])
        nc.sync.dma_start(out=wv_sb, in_=wv.rearrange("(cc p) d -> p cc d", p=128))
        if use_m:
            m_sb = singles.tile([128, 8, 2], F32)
            nc.sync.dma_start(out=m_sb, in_=mvec.rearrange("(rs p) w -> p rs w", p=128))

        identB = singles.tile([128, 128], BF16)
        make_identity(nc, identB)
        identb = singles.tile([65, 65], BF16)
        make_identity(nc, identb)

        qkT_sb = singles.tile([128, 4, S], BF16)     # G-group projection outputs
        v4t_sb = singles.tile([128, 8, HPC * 65], BF16)  # v_aug per r-subtile
        cg_sb = singles.tile([65, HPC, S], BF16)     # unnormalized ctxT, global
        cl_sb = singles.tile([65, HPC, S], BF16)     # local

        # ---- phase P || R: projections + positional bands, one psum scope ----
        with tc.tile_pool(name="ps_pr", bufs=2, space="PSUM") as ps_pr, \
             tc.tile_pool(name="ps_tl", bufs=2, space="PSUM") as ps_tl, \
             tc.tile_pool(name="ps_pv", bufs=2, space="PSUM") as ps_pv, \
             tc.tile_pool(name="rsp", bufs=3) as rsp:

            def emit_G(g):
                mg = 128 if g < 2 else 64
                pt = ps_pr.tile([128, S], F32, tag="big")
                for n in range(2):
                    for cc in range(n_cc):
                        nc.tensor.matmul(
                            pt[:mg, n * 512:(n + 1) * 512],
                            lhsT=wg_sb[:, g, cc, :mg],
                            rhs=hid_sb[:, cc, n * 512:(n + 1) * 512],
                            start=(cc == 0), stop=(cc == n_cc - 1),
                        )
                nc.scalar.activation(qkT_sb[:mg, g, :], pt[:mg], COPY)

            def emit_R(h):
                g, po = QG[h]
                qb = qkT_sb[po:po + 64, g, :]
                for lt in range(LTN):
                    pr = ps_pr.tile([128, S], F32, tag="big")
                    prt = ps_tl.tile([128, BAND - S], F32, tag="tail")
                    e0 = 896 - lt * 128
                    qbl = qb[:, lt * 128:(lt + 1) * 128]
                    for n0, n1 in ((0, 512), (512, 1024)):
                        nc.tensor.matmul(
                            pr[:, n0:n1],
                            lhsT=qbl,
                            rhs=emb_sb[po:po + 64, e0 + n0:e0 + n1],
                            start=True, stop=True,
                        )
                    nc.tensor.matmul(
                        prt, lhsT=qbl,
                        rhs=emb_sb[po:po + 64, e0 + S:e0 + BAND],
                        start=True, stop=True,
                    )
                    rt = rsp.tile([128, BAND], FP8, tag="rt")
                    if lt % 2 == 0:
                        nc.scalar.activation(rt[:, 0:S], pr, COPY)
                        nc.scalar.activation(rt[:, S:BAND], prt, COPY)
                    else:
                        nc.vector.tensor_copy(rt[:, 0:S], pr)
                        nc.vector.tensor_copy(rt[:, S:BAND], prt)
                    wview = skews[h][lt * 128 * PITCH:(lt + 1) * 128 * PITCH] \
                        .rearrange("(p c) -> p c", c=PITCH)[:, 0:BAND]
                    nc.sync.dma_start(out=wview, in_=rt)

            emit_G(0)
            emit_R(0)
            emit_G(1)
            emit_R(1)
            emit_G(2)
            emit_R(2)
            emit_G(3)
            nc.vector.memset(
                v4t_sb.rearrange("p rs (h x) -> p rs h x", x=65)[:, :, :, 64], 1.0
            )
            for rs in range(8):
                pv = ps_pv.tile([128, HPC * HD], F32, tag="pv")
                for cc in range(n_cc):
                    nc.tensor.matmul(
                        pv,
                        lhsT=hid_sb[:, cc, rs * 128:(rs + 1) * 128],
                        rhs=wv_sb[:, cc, :],
                        start=(cc == 0), stop=(cc == n_cc - 1),
                    )
                nc.vector.tensor_copy(
                    v4t_sb[:, rs, :].rearrange("p (h x) -> p h x", x=65)[:, :, 0:64],
                    pv.rearrange("p (h d) -> p h d", d=64),
                )

        nc.sync.dma_start(out=smT_sb, in_=smT.rearrange("(rs p) l -> p rs l", p=128))
        nc.sync.dma_start(out=selw_sb, in_=selw.rearrange("(lc p) w -> p lc w", p=128))

        # ---- phase S: scores + dual softmax + context (+ v projection) ----
        with tc.tile_pool(name="ps_s", bufs=1, space="PSUM") as ps_s, \
             tc.tile_pool(name="ps_bt", bufs=2, space="PSUM") as ps_bt, \
             tc.tile_pool(name="ps_cg", bufs=1, space="PSUM") as ps_cg, \
             tc.tile_pool(name="ps_cl", bufs=1, space="PSUM") as ps_cl, \
             tc.tile_pool(name="wk", bufs=3) as wk, \
             tc.tile_pool(name="bskp", bufs=2) as bskp:

            def emit_S(h):
                bsk8 = bskp.tile([128, LTN, S], FP8, tag="bsk8")
                bsk = bskp.tile([128, LTN, S], BF16, tag="bsk")
                for lt in range(LTN):
                    base = lt * 128 * PITCH
                    rview = skews[h][base + 127:base + 127 + 128 * BAND] \
                        .rearrange("(p c) -> p c", c=BAND)[:, 0:S]
                    nc.sync.dma_start(out=bsk8[:, lt, :], in_=rview)
                    nc.gpsimd.tensor_copy(bsk[:, lt, :], bsk8[:, lt, :])
                qg, qpo = QG[h]
                kg, kpo = KG[h]
                qf = qkT_sb[qpo:qpo + 64, qg, :]
                kf = qkT_sb[kpo:kpo + 64, kg, :]
                cg = ps_cg.tile([65, S], F32, tag="cg")
                cl = ps_cl.tile([65, S], F32, tag="cl")
                for rs in range(8):
                    # positional bias, transposed to [r, l] in a bf16 psum
                    bt = ps_bt.tile([128, S], BF16, tag="bt")
                    for lt in range(LTN):
                        nc.tensor.matmul(
                            bt[:, lt * 128:(lt + 1) * 128],
                            lhsT=bsk[:, lt, rs * 128:(rs + 1) * 128],
                            rhs=identB,
                            is_transpose=True, start=True, stop=True,
                        )
                    # raw scores (q.k), f32 psum
                    st = ps_s.tile([128, S], F32, tag="st")
                    for n in range(2):
                        nc.tensor.matmul(
                            st[:, n * 512:(n + 1) * 512],
                            lhsT=kf[:, rs * 128:(rs + 1) * 128],
                            rhs=qf[:, n * 512:(n + 1) * 512],
                            start=True, stop=True,
                        )
                    if use_m:
                        nc.vector.tensor_scalar_add(st, st, m_sb[:, rs, 1:2])
                    btc = wk.tile([128, S], BF16, tag="btc")
                    if rs % 3 == 2:
                        nc.scalar.activation(btc, bt, COPY)
                    else:
                        nc.vector.tensor_copy(btc, bt)
                    # sgtl = [scores+bias | (scores+bias)*smT], bf16
                    sgtl = wk.tile([128, 2 * S], BF16, tag="sgtl")
                    nc.vector.tensor_add(sgtl[:, 0:S], st, btc)
                    nc.gpsimd.tensor_mul(sgtl[:, S:2 * S], sgtl[:, 0:S],
                                         smT_sb[:, rs, :])
                    pgl = wk.tile([128, 2 * S], BF16, tag="pgl")
                    if use_m:
                        nc.scalar.activation(pgl[:, 0:S], sgtl[:, 0:S], EXP,
                                             scale=0.125)
                        nc.scalar.activation(pgl[:, S:2 * S], sgtl[:, S:2 * S],
                                             EXP, scale=0.125,
                                             bias=m_sb[:, rs, 0:1])
                    else:
                        nc.scalar.activation(pgl, sgtl, EXP, scale=0.125)
                    va = v4t_sb[:, rs, h * 65:(h + 1) * 65]
                    for n in range(2):
                        nc.tensor.matmul(
                            cg[:, n * 512:(n + 1) * 512],
                            lhsT=va, rhs=pgl[:, n * 512:(n + 1) * 512],
                            start=(rs == 0), stop=(rs == 7),
                        )
                        nc.tensor.matmul(
                            cl[:, n * 512:(n + 1) * 512],
                            lhsT=va, rhs=pgl[:, S + n * 512:S + (n + 1) * 512],
                            start=(rs == 0), stop=(rs == 7),
                        )
                nc.scalar.activation(cg_sb[:, h, :], cg, COPY)
                nc.vector.tensor_copy(cl_sb[:, h, :], cl)

            for h in range(HPC):
                emit_S(h)

        # ---- phase F: transpose back, normalize, blend, store ----
        with tc.tile_pool(name="ps_f", bufs=2, space="PSUM") as ps_f, \
             tc.tile_pool(name="fin", bufs=3) as fin:
            for lc in range(LTN):
                pf = ps_f.tile([128, 6 * 66], BF16, tag="pf")
                for h in range(HPC):
                    for br, csb in enumerate((cg_sb, cl_sb)):
                        x = h * 2 + br
                        nc.tensor.matmul(
                            pf[:, x * 66:x * 66 + 65],
                            lhsT=csb[:, h, lc * 128:(lc + 1) * 128],
                            rhs=identb,
                            is_transpose=True, start=True, stop=True,
                        )
                pfv = pf.rearrange("p (x c) -> p x c", c=66)
                rsum = fin.tile([128, 6], F32, tag="rsum")
                nc.vector.reciprocal(rsum, pfv[:, :, 64])
                w = fin.tile([128, 6], F32, tag="w")
                selv = selw_sb[:, lc, :]  # [128, 2]; col0=(1-sel) for g, col1=sel
                nc.vector.tensor_mul(
                    w.rearrange("p (h b) -> p h b", b=2),
                    rsum.rearrange("p (h b) -> p h b", b=2),
                    _bcast(selv, 3, 1),
                )
                tmp = fin.tile([128, 6, 64], F32, tag="tmp")
                nc.vector.tensor_mul(tmp, pfv[:, :, 0:64], _bcast(w, 64, 2))
                osb = fin.tile([128, HPC * HD], F32, tag="osb")
                tv = tmp.rearrange("p (h b) d -> p h b d", b=2)
                nc.vector.tensor_add(
                    osb.rearrange("p (h d) -> p h d", d=64),
                    tv[:, :, 0, :], tv[:, :, 1, :],
                )
                nc.sync.dma_start(out=outp[lc * 128:(lc + 1) * 128, :], in_=osb)

    nc.compile()
    return nc


def _get_program(n_cc, use_m):
    key = (n_cc, use_m)
    if key not in _programs:
        _programs[key] = build_program(n_cc, use_m)
    return _programs[key]


def kernel(hidden_states, attention_mask, scaled_attention_mask, selector_outputs,
           Wq, bq, Wk, bk, Wv, bv, dist_emb):
    hidden_states = np.asarray(hidden_states, np.float32)
    attention_mask = np.asarray(attention_mask, np.float32)
    scaled_attention_mask = np.asarray(scaled_attention_mask, np.float32)
    selector_outputs = np.asarray(selector_outputs, np.float32)
    Wq, Wk, Wv = (np.asarray(x, np.float32) for x in (Wq, Wk, Wv))
    bq, bk, bv = (np.asarray(x, np.float32) for x in (bq, bk, bv))
    dist_emb = np.asarray(dist_emb, np.float32)

    use_bias = bool(np.any(bq) or np.any(bk) or np.any(bv))
    use_m = bool(np.any(attention_mask))
    n_cc = 7 if use_bias else 6
    CH = n_cc * 128
    nc = _get_program(n_cc, use_m)

    smT = np.ascontiguousarray(scaled_attention_mask[0, 0].T).astype(NPBF16)
    e_rev_t = dist_emb[::-1].T.astype(NPBF16)
    embT2 = np.ascontiguousarray(np.concatenate([e_rev_t, e_rev_t], axis=0))

    in_maps = []
    for core in range(NCORES):
        b = core // 4
        k4 = core % 4
        heads = [3 * k4, 3 * k4 + 1, 3 * k4 + 2]

        hidT = hidden_states[b].T  # [768, S]
        if use_bias:
            hidT = np.concatenate(
                [hidT, np.ones((1, S), np.float32),
                 np.zeros((CH - HID - 1, S), np.float32)], axis=0)
        hidT_bf = np.ascontiguousarray(hidT).astype(NPBF16)

        def wcols(W, bvec, h):
            c = W[:, h * HD:(h + 1) * HD]
            if use_bias:
                c = np.concatenate(
                    [c, bvec[None, h * HD:(h + 1) * HD],
                     np.zeros((CH - HID - 1, HD), np.float32)], axis=0)
            return c

        q0, q1, q2 = (wcols(Wq, bq, h) for h in heads)
        k0, k1, k2 = (wcols(Wk, bk, h) for h in heads)
        z = np.zeros_like(q2)
        wg_np = np.stack([
            np.concatenate([q0, q1], axis=1),
            np.concatenate([k0, k1], axis=1),
            np.concatenate([q2, z], axis=1),
            np.concatenate([k2, z], axis=1),
        ]).astype(NPBF16)
        wv_np = np.concatenate(
            [wcols(Wv, bv, h) for h in heads], axis=1).astype(NPBF16)

        sel = selector_outputs[b, 0, :, 0]
        selw_np = np.stack([1.0 - sel, sel], axis=1).astype(np.float32)

        m = {
            "hidT": hidT_bf,
            "wg": wg_np,
            "wv": np.ascontiguousarray(wv_np),
            "embT2": embT2,
            "smT": smT,
            "selw": np.ascontiguousarray(selw_np),
        }
        if use_m:
            mv = attention_mask[b, 0, 0]
            m["mvec"] = np.ascontiguousarray(
                np.stack([mv, 8.0 * mv], axis=1).astype(np.float32))
        in_maps.append(m)

    res = run_bass_kernel_spmd(nc, in_maps, list(range(NCORES)))

    out = np.empty((B, S, HID), np.float32)
    for core in range(NCORES):
        b = core // 4
        k4 = core % 4
        out[b, :, 192 * k4:192 * (k4 + 1)] = res.results[core]["out"]
    return out


# revision 26
# speedup vs baseline: 1.0675x; 1.0675x over previous
"""Bass/Trainium2 kernel for nn_BayesianBertSelfAttention (B=2,S=1024,HID=768,NH=12,HD=64).

Sharding: 24 (batch, head) pairs over 8 cores -> core c handles batch c//4,
heads {3k, 3k+1, 3k+2} with k = c%4.

Per-core device algorithm (transposed-scores layout, scoresT[r, l]):
  phase P: q/k projections as 4 column-packed matmul groups (qT/kT [64, S]
           bf16), v projections directly in [r, d] layout (bf16, with a ones
           column producing softmax row sums via the context matmul).
  phase R (pipelined into S): relative-position table R'[l, c] = q . E_rev
           band per 128-row l-tile (bf16 matmul), copied bf16 to a DRAM
           scratch with row pitch 1152.
  phase S: per head: skewed bf16 read of R' gives bias[l, r] tiles (the
           Music-Transformer skew as a strided DRAM access pattern); PE
           transposes them to [r, l] in a bf16 psum; one DVE add fuses
           psum scores + bias -> bf16 SBUF. Dual softmax: ACT exp(scale=1/8)
           over a fused [128, 2048] global|local pair (local pre-multiplied
           by SM^T on GPSIMD). Unnormalized bf16 probs feed two context
           matmuls; the ones-column of v gives row sums in row 64.
  phase F: PE transposes ctxT back to [l, d], DVE normalizes (reciprocal of
           row sums), blends the two branches with selector weights, DMA out.

Host: packs weights/binds layouts, converts to bf16, reassembles [2,1024,768].
"""

import sys

sys.path.insert(0, "/opt/trn_rl_repo")

import numpy as np
import ml_dtypes
from contextlib import ExitStack

import concourse.bass as bass
import concourse.bacc as bacc
import concourse.tile as tile
from concourse import mybir
from concourse.bass_utils import run_bass_kernel_spmd
from concourse.masks import make_identity

B, S, HID, NH, HD = 2, 1024, 768, 12, 64
MAXP = 1024
NCORES = 8
HPC = 3            # heads per core
LTN = S // 128     # 8 l-tiles
BAND = 1151        # skew band width per 128-row l-tile
PITCH = 1152       # skew scratch row pitch
NE = 2 * MAXP - 1  # 2047

BF16 = mybir.dt.bfloat16
F32 = mybir.dt.float32
FP8 = mybir.dt.float8e4
COPY = mybir.ActivationFunctionType.Copy
EXP = mybir.ActivationFunctionType.Exp

NPBF16 = ml_dtypes.bfloat16

_programs = {}


def _bcast(ap, dim_count, insert_at):
    """Insert a step-0 broadcast dim of size dim_count at free-dim position."""
    new = list(ap.ap)
    new.insert(insert_at, [0, dim_count])
    return bass.AP(tensor=ap.tensor, offset=ap.offset, ap=new)


def build_program(n_cc=6, use_m=False):
    """n_cc: number of 128-row contraction chunks for projections (6 normally,
    7 when biases are nonzero and folded in as an extra ones row)."""
    nc = bacc.Bacc(None)
    CH = n_cc * 128

    hidT = nc.dram_tensor("hidT", [CH, S], BF16, kind="ExternalInput")
    wg = nc.dram_tensor("wg", [4, CH, 128], BF16, kind="ExternalInput")
    wv = nc.dram_tensor("wv", [CH, HPC * HD], BF16, kind="ExternalInput")
    embT2 = nc.dram_tensor("embT2", [128, NE], BF16, kind="ExternalInput")
    smT = nc.dram_tensor("smT", [S, S], BF16, kind="ExternalInput")
    selw = nc.dram_tensor("selw", [S, 2], F32, kind="ExternalInput")
    if use_m:
        mvec = nc.dram_tensor("mvec", [S, 2], F32, kind="ExternalInput")  # [m, 8m]
    outp = nc.dram_tensor("out", [S, HPC * HD], F32, kind="ExternalOutput")
    skews = [nc.dram_tensor(f"skew{h}", [LTN * 128 * PITCH], FP8)
             for h in range(HPC)]

    # (q_h, k_h) SBUF partition offsets per head; G-groups: 0=[q0|q1], 1=[k0|k1],
    # 2=[q2|-], 3=[k2|-]
    QG = [(0, 0), (0, 64), (2, 0)]   # (group, partition offset) for q
    KG = [(1, 0), (1, 64), (3, 0)]

    with tile.TileContext(nc) as tc, ExitStack() as ctx:
        singles = ctx.enter_context(tc.tile_pool(name="singles", bufs=1))

        hid_sb = singles.tile([128, n_cc, S], BF16)
        wg_sb = singles.tile([128, 4, n_cc, 128], BF16)
        wv_sb = singles.tile([128, n_cc, HPC * HD], BF16)
        emb_sb = singles.tile([128, NE], BF16)
        smT_sb = singles.tile([128, 8, S], BF16)
        selw_sb = singles.tile([128, 8, 2], F32)
        hid_v = hidT.rearrange("(cc p) l -> p cc l", p=128)
        wg_v = wg.rearrange("g (cc p) d -> p g cc d", p=128)
        nc.sync.dma_start(out=wg_sb[:, 0], in_=wg_v[:, 0])
        for cc in range(n_cc):
            nc.sync.dma_start(out=hid_sb[:, cc], in_=hid_v[:, cc])
        for g in range(1, 4):
            nc.sync.dma_start(out=wg_sb[:, g], in_=wg_v[:, g])
        nc.sync.dma_start(out=emb_sb, in_=embT2[:, :])
        nc.sync.dma_start(out=wv_sb, in_=wv.rearrange("(cc p) d -> p cc d", p=128))
        if use_m:
            m_sb = singles.tile([128, 8, 2], F32)
            nc.sync.dma_start(out=m_sb, in_=mvec.rearrange("(rs p) w -> p rs w", p=128))

        identB = singles.tile([128, 128], BF16)
        make_identity(nc, identB)
        identb = singles.tile([65, 65], BF16)
        make_identity(nc, identb)

        qkT_sb = singles.tile([128, 4, S], BF16)     # G-group projection outputs
        v4t_sb = singles.tile([128, 8, HPC * 65], BF16)  # v_aug per r-subtile
        cg_sb = singles.tile([65, HPC, S], BF16)     # unnormalized ctxT, global
        cl_sb = singles.tile([65, HPC, S], BF16)     # local

        # ---- phase P || R: projections + positional bands, one psum scope ----
        with tc.tile_pool(name="ps_pr", bufs=2, space="PSUM") as ps_pr, \
             tc.tile_pool(name="ps_tl", bufs=2, space="PSUM") as ps_tl, \
             tc.tile_pool(name="ps_pt", bufs=1, space="PSUM") as ps_pt, \
             tc.tile_pool(name="rsp", bufs=3) as rsp:

            def emit_G(g):
                mg = 128 if g < 2 else 64
                pt = ps_pt.tile([128, S], F32, tag="pt")
                for n in range(2):
                    for cc in range(n_cc):
                        nc.tensor.matmul(
                            pt[:mg, n * 512:(n + 1) * 512],
                            lhsT=wg_sb[:, g, cc, :mg],
                            rhs=hid_sb[:, cc, n * 512:(n + 1) * 512],
                            start=(cc == 0), stop=(cc == n_cc - 1),
                        )
                nc.scalar.activation(qkT_sb[:mg, g, :], pt[:mg], COPY)

            def emit_R(h):
                g, po = QG[h]
                qb = qkT_sb[po:po + 64, g, :]
                for lt in range(LTN):
                    pr = ps_pr.tile([128, S], F32, tag="big")
                    prt = ps_tl.tile([128, BAND - S], F32, tag="tail")
                    e0 = 896 - lt * 128
                    qbl = qb[:, lt * 128:(lt + 1) * 128]
                    for n0, n1 in ((0, 512), (512, 1024)):
                        nc.tensor.matmul(
                            pr[:, n0:n1],
                            lhsT=qbl,
                            rhs=emb_sb[po:po + 64, e0 + n0:e0 + n1],
                            start=True, stop=True,
                        )
                    nc.tensor.matmul(
                        prt, lhsT=qbl,
                        rhs=emb_sb[po:po + 64, e0 + S:e0 + BAND],
                        start=True, stop=True,
                    )
                    rt = rsp.tile([128, BAND], FP8, tag="rt")
                    if lt % 2 == 0:
                        nc.scalar.activation(rt[:, 0:S], pr, COPY)
                        nc.scalar.activation(rt[:, S:BAND], prt, COPY)
                    else:
                        nc.vector.tensor_copy(rt[:, 0:S], pr)
                        nc.vector.tensor_copy(rt[:, S:BAND], prt)
                    wview = skews[h][lt * 128 * PITCH:(lt + 1) * 128 * PITCH] \
                        .rearrange("(p c) -> p c", c=PITCH)[:, 0:BAND]
                    nc.sync.dma_start(out=wview, in_=rt)

            emit_G(0)
            emit_R(0)
            emit_G(1)
            emit_R(1)
            emit_G(2)
            emit_R(2)
            emit_G(3)
            nc.vector.memset(
                v4t_sb.rearrange("p rs (h x) -> p rs h x", x=65)[:, :, :, 64], 1.0
            )
            for rs in range(8):
                pv = ps_pt.tile([128, HPC * HD], F32, tag="pt")
                for cc in range(n_cc):
                    nc.tensor.matmul(
                        pv,
                        lhsT=hid_sb[:, cc, rs * 128:(rs + 1) * 128],
                        rhs=wv_sb[:, cc, :],
                        start=(cc == 0), stop=(cc == n_cc - 1),
                    )
                nc.vector.tensor_copy(
                    v4t_sb[:, rs, :].rearrange("p (h x) -> p h x", x=65)[:, :, 0:64],
                    pv.rearrange("p (h d) -> p h d", d=64),
                )

        nc.sync.dma_start(out=smT_sb, in_=smT.rearrange("(rs p) l -> p rs l", p=128))
        nc.sync.dma_start(out=selw_sb, in_=selw.rearrange("(lc p) w -> p lc w", p=128))

        # ---- phase S: scores + dual softmax + context (+ v projection) ----
        with tc.tile_pool(name="ps_s", bufs=1, space="PSUM") as ps_s, \
             tc.tile_pool(name="ps_bt", bufs=2, space="PSUM") as ps_bt, \
             tc.tile_pool(name="ps_cg", bufs=1, space="PSUM") as ps_cg, \
             tc.tile_pool(name="ps_cl", bufs=1, space="PSUM") as ps_cl, \
             tc.tile_pool(name="wk", bufs=3) as wk, \
             tc.tile_pool(name="bskp", bufs=2) as bskp:

            def emit_S(h):
                bsk8 = bskp.tile([128, LTN, S], FP8, tag="bsk8")
                bsk = bskp.tile([128, LTN, S], BF16, tag="bsk")
                for lt in range(LTN):
                    base = lt * 128 * PITCH
                    rview = skews[h][base + 127:base + 127 + 128 * BAND] \
                        .rearrange("(p c) -> p c", c=BAND)[:, 0:S]
                    nc.sync.dma_start(out=bsk8[:, lt, :], in_=rview)
                    nc.gpsimd.tensor_copy(bsk[:, lt, :], bsk8[:, lt, :])
                qg, qpo = QG[h]
                kg, kpo = KG[h]
                qf = qkT_sb[qpo:qpo + 64, qg, :]
                kf = qkT_sb[kpo:kpo + 64, kg, :]
                cg = ps_cg.tile([65, S], F32, tag="cg")
                cl = ps_cl.tile([65, S], F32, tag="cl")
                for rs in range(8):
                    # positional bias, transposed to [r, l] in a bf16 psum
                    bt = ps_bt.tile([128, S], BF16, tag="bt")
                    for lt in range(LTN):
                        nc.tensor.matmul(
                            bt[:, lt * 128:(lt + 1) * 128],
                            lhsT=bsk[:, lt, rs * 128:(rs + 1) * 128],
                            rhs=identB,
                            is_transpose=True, start=True, stop=True,
                        )
                    # raw scores (q.k), f32 psum
                    st = ps_s.tile([128, S], F32, tag="st")
                    for n in range(2):
                        nc.tensor.matmul(
                            st[:, n * 512:(n + 1) * 512],
                            lhsT=kf[:, rs * 128:(rs + 1) * 128],
                            rhs=qf[:, n * 512:(n + 1) * 512],
                            start=True, stop=True,
                        )
                    if use_m:
                        nc.vector.tensor_scalar_add(st, st, m_sb[:, rs, 1:2])
                    btc = wk.tile([128, S], BF16, tag="btc")
                    if rs % 3 == 2:
                        nc.scalar.activation(btc, bt, COPY)
                    else:
                        nc.vector.tensor_copy(btc, bt)
                    # sgtl = [scores+bias | (scores+bias)*smT], bf16
                    sgtl = wk.tile([128, 2 * S], BF16, tag="sgtl")
                    nc.vector.tensor_add(sgtl[:, 0:S], st, btc)
                    nc.gpsimd.tensor_mul(sgtl[:, S:2 * S], sgtl[:, 0:S],
                                         smT_sb[:, rs, :])
                    pgl = wk.tile([128, 2 * S], BF16, tag="pgl")
                    if use_m:
                        nc.scalar.activation(pgl[:, 0:S], sgtl[:, 0:S], EXP,
                                             scale=0.125)
                        nc.scalar.activation(pgl[:, S:2 * S], sgtl[:, S:2 * S],
                                             EXP, scale=0.125,
                                             bias=m_sb[:, rs, 0:1])
                    else:
                        nc.scalar.activation(pgl, sgtl, EXP, scale=0.125)
                    va = v4t_sb[:, rs, h * 65:(h + 1) * 65]
                    for n in range(2):
                        nc.tensor.matmul(
                            cg[:, n * 512:(n + 1) * 512],
                            lhsT=va, rhs=pgl[:, n * 512:(n + 1) * 512],
                            start=(rs == 0), stop=(rs == 7),
                        )
                        nc.tensor.matmul(
                            cl[:, n * 512:(n + 1) * 512],
                            lhsT=va, rhs=pgl[:, S + n * 512:S + (n + 1) * 512],
                            start=(rs == 0), stop=(rs == 7),
                        )
                nc.scalar.activation(cg_sb[:, h, :], cg, COPY)
                nc.vector.tensor_copy(cl_sb[:, h, :], cl)

            for h in range(HPC):
                emit_S(h)

        # ---- phase F: transpose back, normalize, blend, store ----
        with tc.tile_pool(name="ps_f", bufs=2, space="PSUM") as ps_f, \
             tc.tile_pool(name="fin", bufs=3) as fin:
            for lc in range(LTN):
                pf = ps_f.tile([128, 6 * 66], BF16, tag="pf")
                for h in range(HPC):
                    for br, csb in enumerate((cg_sb, cl_sb)):
                        x = h * 2 + br
                        nc.tensor.matmul(
                            pf[:, x * 66:x * 66 + 65],
                            lhsT=csb[:, h, lc * 128:(lc + 1) * 128],
                            rhs=identb,
                            is_transpose=True, start=True, stop=True,
                        )
                pfv = pf.rearrange("p (x c) -> p x c", c=66)
                rsum = fin.tile([128, 6], F32, tag="rsum")
                nc.vector.reciprocal(rsum, pfv[:, :, 64])
                w = fin.tile([128, 6], F32, tag="w")
                selv = selw_sb[:, lc, :]  # [128, 2]; col0=(1-sel) for g, col1=sel
                nc.vector.tensor_mul(
                    w.rearrange("p (h b) -> p h b", b=2),
                    rsum.rearrange("p (h b) -> p h b", b=2),
                    _bcast(selv, 3, 1),
                )
                tmp = fin.tile([128, 6, 64], F32, tag="tmp")
                nc.vector.tensor_mul(tmp, pfv[:, :, 0:64], _bcast(w, 64, 2))
                osb = fin.tile([128, HPC * HD], F32, tag="osb")
                tv = tmp.rearrange("p (h b) d -> p h b d", b=2)
                nc.vector.tensor_add(
                    osb.rearrange("p (h d) -> p h d", d=64),
                    tv[:, :, 0, :], tv[:, :, 1, :],
                )
                nc.sync.dma_start(out=outp[lc * 128:(lc + 1) * 128, :], in_=osb)

    nc.compile()
    return nc


def _get_program(n_cc, use_m):
    key = (n_cc, use_m)
    if key not in _programs:
        _programs[key] = build_program(n_cc, use_m)
    return _programs[key]


def kernel(hidden_states, attention_mask, scaled_attention_mask, selector_outputs,
           Wq, bq, Wk, bk, Wv, bv, dist_emb):
    hidden_states = np.asarray(hidden_states, np.float32)
    attention_mask = np.asarray(attention_mask, np.float32)
    scaled_attention_mask = np.asarray(scaled_attention_mask, np.float32)
    selector_outputs = np.asarray(selector_outputs, np.float32)
    Wq, Wk, Wv = (np.asarray(x, np.float32) for x in (Wq, Wk, Wv))
    bq, bk, bv = (np.asarray(x, np.float32) for x in (bq, bk, bv))
    dist_emb = np.asarray(dist_emb, np.float32)

    use_bias = bool(np.any(bq) or np.any(bk) or np.any(bv))
    use_m = bool(np.any(attention_mask))
    n_cc = 7 if use_bias else 6
    CH = n_cc * 128
    nc = _get_program(n_cc, use_m)

    smT = np.ascontiguousarray(scaled_attention_mask[0, 0].T).astype(NPBF16)
    e_rev_t = dist_emb[::-1].T.astype(NPBF16)
    embT2 = np.ascontiguousarray(np.concatenate([e_rev_t, e_rev_t], axis=0))

    in_maps = []
    for core in range(NCORES):
        b = core // 4
        k4 = core % 4
        heads = [3 * k4, 3 * k4 + 1, 3 * k4 + 2]

        hidT = hidden_states[b].T  # [768, S]
        if use_bias:
            hidT = np.concatenate(
                [hidT, np.ones((1, S), np.float32),
                 np.zeros((CH - HID - 1, S), np.float32)], axis=0)
        hidT_bf = np.ascontiguousarray(hidT).astype(NPBF16)

        def wcols(W, bvec, h):
            c = W[:, h * HD:(h + 1) * HD]
            if use_bias:
                c = np.concatenate(
                    [c, bvec[None, h * HD:(h + 1) * HD],
                     np.zeros((CH - HID - 1, HD), np.float32)], axis=0)
            return c

        q0, q1, q2 = (wcols(Wq, bq, h) for h in heads)
        k0, k1, k2 = (wcols(Wk, bk, h) for h in heads)
        z = np.zeros_like(q2)
        wg_np = np.stack([
            np.concatenate([q0, q1], axis=1),
            np.concatenate([k0, k1], axis=1),
            np.concatenate([q2, z], axis=1),
            np.concatenate([k2, z], axis=1),
        ]).astype(NPBF16)
        wv_np = np.concatenate(
            [wcols(Wv, bv, h) for h in heads], axis=1).astype(NPBF16)

        sel = selector_outputs[b, 0, :, 0]
        selw_np = np.stack([1.0 - sel, sel], axis=1).astype(np.float32)

        m = {
            "hidT": hidT_bf,
            "wg": wg_np,
            "wv": np.ascontiguousarray(wv_np),
            "embT2": embT2,
            "smT": smT,
            "selw": np.ascontiguousarray(selw_np),
        }
        if use_m:
            mv = attention_mask[b, 0, 0]
            m["mvec"] = np.ascontiguousarray(
                np.stack([mv, 8.0 * mv], axis=1).astype(np.float32))
        in_maps.append(m)

    res = run_bass_kernel_spmd(nc, in_maps, list(range(NCORES)))

    out = np.empty((B, S, HID), np.float32)
    for core in range(NCORES):
        b = core // 4
        k4 = core % 4
        out[b, :, 192 * k4:192 * (k4 + 1)] = res.results[core]["out"]
    return out


# revision 29
# speedup vs baseline: 1.0888x; 1.0199x over previous
"""Bass/Trainium2 kernel for nn_BayesianBertSelfAttention (B=2,S=1024,HID=768,NH=12,HD=64).

Sharding: 24 (batch, head) pairs over 8 cores -> core c handles batch c//4,
heads {3k, 3k+1, 3k+2} with k = c%4.

Per-core device algorithm (transposed-scores layout, scoresT[r, l]):
  phase P: q/k projections as 4 column-packed matmul groups (qT/kT [64, S]
           bf16), v projections directly in [r, d] layout (bf16, with a ones
           column producing softmax row sums via the context matmul).
  phase R (pipelined into S): relative-position table R'[l, c] = q . E_rev
           band per 128-row l-tile (bf16 matmul), copied bf16 to a DRAM
           scratch with row pitch 1152.
  phase S: per head: skewed bf16 read of R' gives bias[l, r] tiles (the
           Music-Transformer skew as a strided DRAM access pattern); PE
           transposes them to [r, l] in a bf16 psum; one DVE add fuses
           psum scores + bias -> bf16 SBUF. Dual softmax: ACT exp(scale=1/8)
           over a fused [128, 2048] global|local pair (local pre-multiplied
           by SM^T on GPSIMD). Unnormalized bf16 probs feed two context
           matmuls; the ones-column of v gives row sums in row 64.
  phase F: PE transposes ctxT back to [l, d], DVE normalizes (reciprocal of
           row sums), blends the two branches with selector weights, DMA out.

Host: packs weights/binds layouts, converts to bf16, reassembles [2,1024,768].
"""

import sys

sys.path.insert(0, "/opt/trn_rl_repo")

import numpy as np
import ml_dtypes
from contextlib import ExitStack

import concourse.bass as bass
import concourse.bacc as bacc
import concourse.tile as tile
from concourse import mybir
from concourse.bass_utils import run_bass_kernel_spmd
from concourse.masks import make_identity

B, S, HID, NH, HD = 2, 1024, 768, 12, 64
MAXP = 1024
NCORES = 8
HPC = 3            # heads per core
LTN = S // 128     # 8 l-tiles
BAND = 1151        # skew band width per 128-row l-tile
PITCH = 1152       # skew scratch row pitch
NE = 2 * MAXP - 1  # 2047

BF16 = mybir.dt.bfloat16
F32 = mybir.dt.float32
FP8 = mybir.dt.float8e4
COPY = mybir.ActivationFunctionType.Copy
EXP = mybir.ActivationFunctionType.Exp

NPBF16 = ml_dtypes.bfloat16

_programs = {}


def _bcast(ap, dim_count, insert_at):
    """Insert a step-0 broadcast dim of size dim_count at free-dim position."""
    new = list(ap.ap)
    new.insert(insert_at, [0, dim_count])
    return bass.AP(tensor=ap.tensor, offset=ap.offset, ap=new)


def build_program(n_cc=6, use_m=False):
    """n_cc: number of 128-row contraction chunks for projections (6 normally,
    7 when biases are nonzero and folded in as an extra ones row)."""
    nc = bacc.Bacc(None)
    CH = n_cc * 128

    hidT = nc.dram_tensor("hidT", [CH, S], BF16, kind="ExternalInput")
    wg = nc.dram_tensor("wg", [4, CH, 128], BF16, kind="ExternalInput")
    wv = nc.dram_tensor("wv", [CH, HPC * HD], BF16, kind="ExternalInput")
    embT2 = nc.dram_tensor("embT2", [128, NE], BF16, kind="ExternalInput")
    smT = nc.dram_tensor("smT", [S, S], BF16, kind="ExternalInput")
    selw = nc.dram_tensor("selw", [S, 2], F32, kind="ExternalInput")
    if use_m:
        mvec = nc.dram_tensor("mvec", [S, 2], F32, kind="ExternalInput")  # [m, 8m]
    outp = nc.dram_tensor("out", [S, HPC * HD], F32, kind="ExternalOutput")
    skews = [nc.dram_tensor(f"skew{h}", [LTN * 128 * PITCH], FP8)
             for h in range(HPC)]

    # (q_h, k_h) SBUF partition offsets per head; G-groups: 0=[q0|q1], 1=[k0|k1],
    # 2=[q2|-], 3=[k2|-]
    QG = [(0, 0), (0, 64), (2, 0)]   # (group, partition offset) for q
    KG = [(1, 0), (1, 64), (3, 0)]

    with tile.TileContext(nc) as tc, ExitStack() as ctx:
        singles = ctx.enter_context(tc.tile_pool(name="singles", bufs=1))

        hid_sb = singles.tile([128, n_cc, S], BF16)
        wg_sb = singles.tile([128, 4, n_cc, 128], BF16)
        wv_sb = singles.tile([128, n_cc, HPC * HD], BF16)
        emb_sb = singles.tile([128, NE], BF16)
        smT_sb = singles.tile([128, 8, S], BF16)
        selw_sb = singles.tile([128, 8, 2], F32)
        hid_v = hidT.rearrange("(cc p) l -> p cc l", p=128)
        wg_v = wg.rearrange("g (cc p) d -> p g cc d", p=128)
        nc.sync.dma_start(out=wg_sb[:, 0], in_=wg_v[:, 0])
        for cc in range(n_cc):
            nc.sync.dma_start(out=hid_sb[:, cc], in_=hid_v[:, cc])
        for g in range(1, 4):
            nc.sync.dma_start(out=wg_sb[:, g], in_=wg_v[:, g])
        nc.sync.dma_start(out=emb_sb, in_=embT2[:, :])
        nc.sync.dma_start(out=wv_sb, in_=wv.rearrange("(cc p) d -> p cc d", p=128))
        if use_m:
            m_sb = singles.tile([128, 8, 2], F32)
            nc.sync.dma_start(out=m_sb, in_=mvec.rearrange("(rs p) w -> p rs w", p=128))

        identB = singles.tile([128, 128], BF16)
        make_identity(nc, identB)
        identb = singles.tile([65, 65], BF16)
        make_identity(nc, identb)

        qkT_sb = singles.tile([128, 4, S], BF16)     # G-group projection outputs
        v4t_sb = singles.tile([128, 8, HPC * 65], BF16)  # v_aug per r-subtile
        cg_sb = singles.tile([65, HPC, S], BF16)     # unnormalized ctxT, global
        cl_sb = singles.tile([65, HPC, S], BF16)     # local

        # ---- phase P || R: projections + positional bands, one psum scope ----
        with tc.tile_pool(name="ps_pr", bufs=2, space="PSUM") as ps_pr, \
             tc.tile_pool(name="ps_tl", bufs=2, space="PSUM") as ps_tl, \
             tc.tile_pool(name="ps_pt", bufs=1, space="PSUM") as ps_pt, \
             tc.tile_pool(name="rsp", bufs=3) as rsp:

            def emit_G(g):
                mg = 128 if g < 2 else 64
                pt = ps_pt.tile([128, S], F32, tag="pt")
                for n in range(2):
                    for cc in range(n_cc):
                        nc.tensor.matmul(
                            pt[:mg, n * 512:(n + 1) * 512],
                            lhsT=wg_sb[:, g, cc, :mg],
                            rhs=hid_sb[:, cc, n * 512:(n + 1) * 512],
                            start=(cc == 0), stop=(cc == n_cc - 1),
                        )
                nc.scalar.activation(qkT_sb[:mg, g, :], pt[:mg], COPY)

            def emit_R(h):
                g, po = QG[h]
                qb = qkT_sb[po:po + 64, g, :]
                for lt in range(LTN):
                    pr = ps_pr.tile([128, S], F32, tag="big")
                    prt = ps_tl.tile([128, BAND - S], F32, tag="tail")
                    e0 = 896 - lt * 128
                    qbl = qb[:, lt * 128:(lt + 1) * 128]
                    for n0, n1 in ((0, 512), (512, 1024)):
                        nc.tensor.matmul(
                            pr[:, n0:n1],
                            lhsT=qbl,
                            rhs=emb_sb[po:po + 64, e0 + n0:e0 + n1],
                            start=True, stop=True,
                        )
                    nc.tensor.matmul(
                        prt, lhsT=qbl,
                        rhs=emb_sb[po:po + 64, e0 + S:e0 + BAND],
                        start=True, stop=True,
                    )
                    rt = rsp.tile([128, BAND], FP8, tag="rt")
                    if lt % 2 == 0:
                        nc.scalar.activation(rt[:, 0:S], pr, COPY)
                        nc.scalar.activation(rt[:, S:BAND], prt, COPY)
                    else:
                        nc.vector.tensor_copy(rt[:, 0:S], pr)
                        nc.vector.tensor_copy(rt[:, S:BAND], prt)
                    wview = skews[h][lt * 128 * PITCH:(lt + 1) * 128 * PITCH] \
                        .rearrange("(p c) -> p c", c=PITCH)[:, 0:BAND]
                    nc.sync.dma_start(out=wview, in_=rt)

            emit_G(0)
            emit_R(0)
            emit_G(1)
            emit_R(1)
            emit_G(2)
            emit_R(2)
            emit_G(3)
            nc.vector.memset(
                v4t_sb.rearrange("p rs (h x) -> p rs h x", x=65)[:, :, :, 64], 1.0
            )
            for rs in range(8):
                pv = ps_pt.tile([128, HPC * HD], F32, tag="pt")
                for cc in range(n_cc):
                    nc.tensor.matmul(
                        pv,
                        lhsT=hid_sb[:, cc, rs * 128:(rs + 1) * 128],
                        rhs=wv_sb[:, cc, :],
                        start=(cc == 0), stop=(cc == n_cc - 1),
                    )
                nc.vector.tensor_copy(
                    v4t_sb[:, rs, :].rearrange("p (h x) -> p h x", x=65)[:, :, 0:64],
                    pv.rearrange("p (h d) -> p h d", d=64),
                )

        smT_v = smT.rearrange("(rs p) l -> p rs l", p=128)
        for rs in range(8):
            nc.sync.dma_start(out=smT_sb[:, rs], in_=smT_v[:, rs])
        nc.sync.dma_start(out=selw_sb, in_=selw.rearrange("(lc p) w -> p lc w", p=128))

        # ---- phase S: scores + dual softmax + context (+ v projection) ----
        with tc.tile_pool(name="ps_s", bufs=1, space="PSUM") as ps_s, \
             tc.tile_pool(name="ps_bt", bufs=2, space="PSUM") as ps_bt, \
             tc.tile_pool(name="ps_cg", bufs=1, space="PSUM") as ps_cg, \
             tc.tile_pool(name="ps_cl", bufs=1, space="PSUM") as ps_cl, \
             tc.tile_pool(name="wk", bufs=3) as wk, \
             tc.tile_pool(name="bskp", bufs=2) as bskp:

            def emit_S(h):
                bsk8 = bskp.tile([128, LTN, S], FP8, tag="bsk8")
                bsk = bskp.tile([128, LTN, S], BF16, tag="bsk")
                for lt in range(LTN):
                    base = lt * 128 * PITCH
                    rview = skews[h][base + 127:base + 127 + 128 * BAND] \
                        .rearrange("(p c) -> p c", c=BAND)[:, 0:S]
                    nc.sync.dma_start(out=bsk8[:, lt, :], in_=rview)
                    nc.gpsimd.tensor_copy(bsk[:, lt, :], bsk8[:, lt, :])
                qg, qpo = QG[h]
                kg, kpo = KG[h]
                qf = qkT_sb[qpo:qpo + 64, qg, :]
                kf = qkT_sb[kpo:kpo + 64, kg, :]
                cg = ps_cg.tile([65, S], F32, tag="cg")
                cl = ps_cl.tile([65, S], F32, tag="cl")
                for rs in range(8):
                    # positional bias, transposed to [r, l] in a bf16 psum
                    bt = ps_bt.tile([128, S], BF16, tag="bt")
                    for lt in range(LTN):
                        nc.tensor.matmul(
                            bt[:, lt * 128:(lt + 1) * 128],
                            lhsT=bsk[:, lt, rs * 128:(rs + 1) * 128],
                            rhs=identB,
                            is_transpose=True, start=True, stop=True,
                        )
                    # raw scores (q.k), f32 psum
                    st = ps_s.tile([128, S], F32, tag="st")
                    for n in range(2):
                        nc.tensor.matmul(
                            st[:, n * 512:(n + 1) * 512],
                            lhsT=kf[:, rs * 128:(rs + 1) * 128],
                            rhs=qf[:, n * 512:(n + 1) * 512],
                            start=True, stop=True,
                        )
                    if use_m:
                        nc.vector.tensor_scalar_add(st, st, m_sb[:, rs, 1:2])
                    btc = wk.tile([128, S], BF16, tag="btc")
                    if rs % 3 == 2:
                        nc.scalar.activation(btc, bt, COPY)
                    else:
                        nc.vector.tensor_copy(btc, bt)
                    # sgtl = [scores+bias | (scores+bias)*smT], bf16
                    sgtl = wk.tile([128, 2 * S], BF16, tag="sgtl")
                    nc.vector.tensor_add(sgtl[:, 0:S], st, btc)
                    nc.gpsimd.tensor_mul(sgtl[:, S:2 * S], sgtl[:, 0:S],
                                         smT_sb[:, rs, :])
                    pgl = wk.tile([128, 2 * S], BF16, tag="pgl")
                    if use_m:
                        nc.scalar.activation(pgl[:, 0:S], sgtl[:, 0:S], EXP,
                                             scale=0.125)
                        nc.scalar.activation(pgl[:, S:2 * S], sgtl[:, S:2 * S],
                                             EXP, scale=0.125,
                                             bias=m_sb[:, rs, 0:1])
                    else:
                        nc.scalar.activation(pgl, sgtl, EXP, scale=0.125)
                    va = v4t_sb[:, rs, h * 65:(h + 1) * 65]
                    for n in range(2):
                        nc.tensor.matmul(
                            cg[:, n * 512:(n + 1) * 512],
                            lhsT=va, rhs=pgl[:, n * 512:(n + 1) * 512],
                            start=(rs == 0), stop=(rs == 7),
                        )
                        nc.tensor.matmul(
                            cl[:, n * 512:(n + 1) * 512],
                            lhsT=va, rhs=pgl[:, S + n * 512:S + (n + 1) * 512],
                            start=(rs == 0), stop=(rs == 7),
                        )
                nc.scalar.activation(cg_sb[:, h, :], cg, COPY)
                nc.vector.tensor_copy(cl_sb[:, h, :], cl)

            for h in range(HPC):
                emit_S(h)

        # ---- phase F: transpose back, normalize, blend, store ----
        with tc.tile_pool(name="ps_f", bufs=2, space="PSUM") as ps_f, \
             tc.tile_pool(name="fin", bufs=3) as fin:
            for lc in range(LTN):
                pf = ps_f.tile([128, 6 * 66], BF16, tag="pf")
                for h in range(HPC):
                    for br, csb in enumerate((cg_sb, cl_sb)):
                        x = h * 2 + br
                        nc.tensor.matmul(
                            pf[:, x * 66:x * 66 + 65],
                            lhsT=csb[:, h, lc * 128:(lc + 1) * 128],
                            rhs=identb,
                            is_transpose=True, start=True, stop=True,
                        )
                pfv = pf.rearrange("p (x c) -> p x c", c=66)
                rsum = fin.tile([128, 6], F32, tag="rsum")
                nc.vector.reciprocal(rsum, pfv[:, :, 64])
                w = fin.tile([128, 6], F32, tag="w")
                selv = selw_sb[:, lc, :]  # [128, 2]; col0=(1-sel) for g, col1=sel
                nc.vector.tensor_mul(
                    w.rearrange("p (h b) -> p h b", b=2),
                    rsum.rearrange("p (h b) -> p h b", b=2),
                    _bcast(selv, 3, 1),
                )
                tmp = fin.tile([128, 6, 64], F32, tag="tmp")
                nc.vector.tensor_mul(tmp, pfv[:, :, 0:64], _bcast(w, 64, 2))
                osb = fin.tile([128, HPC * HD], F32, tag="osb")
                tv = tmp.rearrange("p (h b) d -> p h b d", b=2)
                nc.vector.tensor_add(
                    osb.rearrange("p (h d) -> p h d", d=64),
                    tv[:, :, 0, :], tv[:, :, 1, :],
                )
                nc.sync.dma_start(out=outp[lc * 128:(lc + 1) * 128, :], in_=osb)

    nc.compile()
    return nc


def _get_program(n_cc, use_m):
    key = (n_cc, use_m)
    if key not in _programs:
        _programs[key] = build_program(n_cc, use_m)
    return _programs[key]


def kernel(hidden_states, attention_mask, scaled_attention_mask, selector_outputs,
           Wq, bq, Wk, bk, Wv, bv, dist_emb):
    hidden_states = np.asarray(hidden_states, np.float32)
    attention_mask = np.asarray(attention_mask, np.float32)
    scaled_attention_mask = np.asarray(scaled_attention_mask, np.float32)
    selector_outputs = np.asarray(selector_outputs, np.float32)
    Wq, Wk, Wv = (np.asarray(x, np.float32) for x in (Wq, Wk, Wv))
    bq, bk, bv = (np.asarray(x, np.float32) for x in (bq, bk, bv))
    dist_emb = np.asarray(dist_emb, np.float32)

    use_bias = bool(np.any(bq) or np.any(bk) or np.any(bv))
    use_m = bool(np.any(attention_mask))
    n_cc = 7 if use_bias else 6
    CH = n_cc * 128
    nc = _get_program(n_cc, use_m)

    smT = np.ascontiguousarray(scaled_attention_mask[0, 0].T).astype(NPBF16)
    e_rev_t = dist_emb[::-1].T.astype(NPBF16)
    embT2 = np.ascontiguousarray(np.concatenate([e_rev_t, e_rev_t], axis=0))

    in_maps = []
    for core in range(NCORES):
        b = core // 4
        k4 = core % 4
        heads = [3 * k4, 3 * k4 + 1, 3 * k4 + 2]

        hidT = hidden_states[b].T  # [768, S]
        if use_bias:
            hidT = np.concatenate(
                [hidT, np.ones((1, S), np.float32),
                 np.zeros((CH - HID - 1, S), np.float32)], axis=0)
        hidT_bf = np.ascontiguousarray(hidT).astype(NPBF16)

        def wcols(W, bvec, h):
            c = W[:, h * HD:(h + 1) * HD]
            if use_bias:
                c = np.concatenate(
                    [c, bvec[None, h * HD:(h + 1) * HD],
                     np.zeros((CH - HID - 1, HD), np.float32)], axis=0)
            return c

        q0, q1, q2 = (wcols(Wq, bq, h) for h in heads)
        k0, k1, k2 = (wcols(Wk, bk, h) for h in heads)
        z = np.zeros_like(q2)
        wg_np = np.stack([
            np.concatenate([q0, q1], axis=1),
            np.concatenate([k0, k1], axis=1),
            np.concatenate([q2, z], axis=1),
            np.concatenate([k2, z], axis=1),
        ]).astype(NPBF16)
        wv_np = np.concatenate(
            [wcols(Wv, bv, h) for h in heads], axis=1).astype(NPBF16)

        sel = selector_outputs[b, 0, :, 0]
        selw_np = np.stack([1.0 - sel, sel], axis=1).astype(np.float32)

        m = {
            "hidT": hidT_bf,
            "wg": wg_np,
            "wv": np.ascontiguousarray(wv_np),
            "embT2": embT2,
            "smT": smT,
            "selw": np.ascontiguousarray(selw_np),
        }
        if use_m:
            mv = attention_mask[b, 0, 0]
            m["mvec"] = np.ascontiguousarray(
                np.stack([mv, 8.0 * mv], axis=1).astype(np.float32))
        in_maps.append(m)

    res = run_bass_kernel_spmd(nc, in_maps, list(range(NCORES)))

    out = np.empty((B, S, HID), np.float32)
    for core in range(NCORES):
        b = core // 4
        k4 = core % 4
        out[b, :, 192 * k4:192 * (k4 + 1)] = res.results[core]["out"]
    return out
